# revision 8
# baseline (speedup 1.0000x reference)
"""Trainium2 Bass kernel for nn_Conv4D: 4D conv with separable 3x3x3x3 kernel.

Math: for each batch b, with X[b] = x[b].reshape(64, 64) (rows = (d1,d2) flat,
cols = (d3,d4) flat), the output is

    out[b, i'j', k'l'] = sum_{c,d in 3x3} (K[c,d] * W)^T @ X[b][:, window(c,d)]

where W[ (i'+a)*8 + (j'+e), i'*6+j' ] = K[a,e] is the 64->36 banded matrix of
the (d1,d2)-conv, and window(c,d) selects the shifted 6x6 (d3,d4) patch.  The
(d3,d4)-conv becomes 9 PSUM-accumulated matmuls against shifted free-dim views
of the same SBUF tile -- no transposes anywhere.

Batch packing: 2 batches stack on the 128 partitions (partition = 64*h + ij).
Block-diagonal weights [ [Wcd, 0], [0, Wcd] ] (128x72) route each batch's 64
ij-rows to its own 36 output partitions.  K=128, M=72, N = 14 pairs * 36 =
504 <= 512 (moving-operand max; one PSUM bank).

DMA layout: host-side prep is free, so the input is pre-shuffled ON HOST into
the exact SBUF image and downcast to bf16 (tolerance 2e-2; bf16 costs ~4e-3):
per-partition DMA runs are multi-KB (vs 256B rows -> packet-bound at 120GB/s).

Free-dim order: within a supergroup the SBUF image is [p][k][l][n] with the
PAIR index n innermost (contiguous run of SUPER_PAIRS) so the PE's AP walker
streams long runs; the shifted (k,l) window only crosses dims every SUPER_PAIRS
rows instead of every 6 (measured 33% row-rate penalty the other way).  PSUM
columns come out in (k',l',n) order; the host decode unpermutes.

PSUM->SBUF copies alternate between the Scalar and Vector engines (one engine
alone is ~1.3us/copy x 37 copies = half the kernel span).

Sharding: pure data parallelism, batch dim split across 8 cores (1024 each).
"""

import numpy as np
import ml_dtypes

import concourse.bass as bass
import concourse.bacc as bacc
import concourse.mybir as mybir
from concourse.tile import TileContext
from concourse.bass_utils import run_bass_kernel_spmd

N_CORES = 8
B = 8192
B_C = B // N_CORES            # 1024 batches per core
PAIRS = B_C // 2              # 512 batch pairs per core
PAIRS_PER_GROUP = 14          # N = 14*36 = 504 <= 512 (moving-operand max)
GPG = 4                       # groups per supergroup (one in-DMA / out-DMA)
SUPER = GPG * PAIRS_PER_GROUP # 56 pairs = 112 batches
BF16 = mybir.dt.bfloat16
F32 = mybir.dt.float32
NP_BF16 = ml_dtypes.bfloat16

SHIFTS = [(c, d) for c in range(3) for d in range(3)]


def _super_sizes():
    sizes = []
    left = PAIRS
    while left > 0:
        n = min(SUPER, left)
        sizes.append(n)
        left -= n
    return sizes


def build_w_stack(kern: np.ndarray) -> np.ndarray:
    """Host-side prep of the 9 block-diagonal stationary matrices from the
    raw 3x3 kernel (9 floats -> 128x648 bf16; tiny next to the 8 MiB input).
    """
    kern = np.asarray(kern, np.float32)
    W = np.zeros((64, 36), np.float32)
    for ip in range(6):
        for jp in range(6):
            m = ip * 6 + jp
            for a in range(3):
                for e in range(3):
                    W[(ip + a) * 8 + (jp + e), m] = kern[a, e]
    wstack = np.zeros((128, 9 * 72), np.float32)
    for s, (c, d) in enumerate(SHIFTS):
        wcd = kern[c, d] * W
        wstack[0:64, s * 72 : s * 72 + 36] = wcd
        wstack[64:128, s * 72 + 36 : s * 72 + 72] = wcd
    return wstack.astype(NP_BF16)


_PROGRAM_CACHE = {}


def build_program() -> bass.Bass:
    if "nc" in _PROGRAM_CACHE:
        return _PROGRAM_CACHE["nc"]

    # Bacc (not raw Bass): its compile()/finalize() runs
    # move_matmul_waits_to_ldweights + generate_event_semaphores, which split
    # multi-wait instructions (TRN2 allows 1 sync wait per instruction).
    nc = bacc.Bacc()
    x = nc.dram_tensor("x", [128, PAIRS * 64], BF16, kind="ExternalInput")
    w = nc.dram_tensor("w", [128, 9 * 72], BF16, kind="ExternalInput")
    o = nc.dram_tensor("o", [72, PAIRS * 36], BF16, kind="ExternalOutput")

    with TileContext(nc) as tc:
        with (
            tc.tile_pool(name="wp", bufs=1) as wp,
            tc.tile_pool(name="xp", bufs=3) as xp,
            tc.tile_pool(name="pp", bufs=6, space="PSUM") as pp,
            tc.tile_pool(name="op", bufs=3) as op,
        ):
            wt = wp.tile([128, 9 * 72], BF16)
            nc.sync.dma_start(out=wt[:, :], in_=w[:, :])

            gidx = 0
            pcur = 0  # pair cursor
            for spairs in _super_sizes():
                xg = xp.tile([128, SUPER * 64], BF16, tag="xg")
                nc.sync.dma_start(
                    out=xg[:, : spairs * 64],
                    in_=x[:, pcur * 64 : (pcur + spairs) * 64],
                )
                ot = op.tile([72, SUPER * 36], BF16, tag="ot")

                done = 0
                while done < spairs:
                    npair = min(PAIRS_PER_GROUP, spairs - done)
                    nfree = npair * 36

                    ps = pp.tile([72, PAIRS_PER_GROUP * 36], F32, tag="ps")
                    # Gate matmul: absorbs the psum-slot-release (and, for
                    # group 0, the weight-DMA) wait so each real matmul
                    # carries at most one sync wait.
                    nc.tensor.matmul(
                        ps[0:2, 0:2], wt[:, 0:2], wt[:, 0:2], start=True, stop=True
                    )
                    # SBUF image within a supergroup: [p][k(8)][l(8)][n].
                    # rhs free dims iterate (k',l',n) with n innermost and
                    # contiguous; psum columns land as k'*6*npair + l'*npair + n.
                    xv = xg[:, : spairs * 64].rearrange(
                        "p (k l n) -> p k l n", k=8, l=8
                    )
                    for s, (c, d) in enumerate(SHIFTS):
                        nc.tensor.matmul(
                            ps[:, :nfree],
                            wt[:, s * 72 : (s + 1) * 72],
                            xv[:, c : c + 6, d : d + 6, done : done + npair],
                            start=(s == 0),
                            stop=(s == len(SHIFTS) - 1),
                        )

                    dst = ot[:, done * 36 : done * 36 + nfree]
                    if gidx % 2 == 0:
                        nc.scalar.copy(out=dst, in_=ps[:, :nfree])
                    else:
                        nc.vector.tensor_copy(out=dst, in_=ps[:, :nfree])
                    done += npair
                    gidx += 1

                nc.sync.dma_start(
                    out=o[:, pcur * 36 : (pcur + spairs) * 36],
                    in_=ot[:, : spairs * 36],
                )
                pcur += spairs

    # Bacc.finalize runs compile() (register alloc, wait splitting via event
    # semaphores) then freezes; the PJRT exec path requires a finalized nc.
    nc.finalize()

    _PROGRAM_CACHE["nc"] = nc
    return nc


def shard_inputs(input_tensor: np.ndarray, kern: np.ndarray):
    """Host prep: shuffle each core's slice into the SBUF image and downcast.

    Per supergroup of S pairs starting at pair P: partition p = 64*h + ij
    holds x[2*(P+n)+h, ij, k*8+l] at free offset P*64 + (k*8+l)*S + n.
    """
    x = np.ascontiguousarray(np.asarray(input_tensor, np.float32))
    xs = x.reshape(N_CORES, PAIRS, 2, 64, 64)  # (core, pair, h, ij, kl)
    wstack = build_w_stack(kern)
    in_maps = []
    for c in range(N_CORES):
        blocks = []
        pcur = 0
        for spairs in _super_sizes():
            blk = xs[c][pcur : pcur + spairs]          # (S, 2, 64, 64)
            blk = blk.transpose(1, 2, 3, 0)            # (2, ij, kl, S)
            blocks.append(blk.reshape(128, spairs * 64))
            pcur += spairs
        xd = np.concatenate(blocks, axis=1).astype(NP_BF16)
        in_maps.append({"x": np.ascontiguousarray(xd), "w": wstack})
    return in_maps


def unshard_output(results) -> np.ndarray:
    """o[36*h+ij', P*36 + G*36*g + kl'*npair + n] -> out[b, i',j',k',l']."""
    outs = []
    for r in results:
        od = np.asarray(r["o"]).astype(np.float32)  # (72, PAIRS*36)
        out = np.empty((B_C, 36, 36), np.float32)
        pcur = 0
        for spairs in _super_sizes():
            done = 0
            while done < spairs:
                npair = min(PAIRS_PER_GROUP, spairs - done)
                col0 = (pcur + done) * 36
                blk = od[:, col0 : col0 + npair * 36]
                # (2, ij', 36 kl', npair) -> (npair, 2, ij', kl')
                blk = blk.reshape(2, 36, 36, npair).transpose(3, 0, 1, 2)
                b0 = (pcur + done) * 2
                out[b0 : b0 + 2 * npair] = blk.reshape(2 * npair, 36, 36)
                done += npair
            pcur += spairs
        outs.append(out.reshape(B_C, 6, 6, 6, 6))
    return np.concatenate(outs, axis=0)


def run(input_tensor: np.ndarray, kern: np.ndarray, **spmd_kwargs):
    """Shard, run on 8 cores, gather.  Returns (output, BassKernelResults)."""
    in_maps = shard_inputs(input_tensor, kern)
    nc = build_program()
    res = run_bass_kernel_spmd(nc, in_maps, core_ids=list(range(N_CORES)), **spmd_kwargs)
    return unshard_output(res.results), res


def kernel(input_tensor: np.ndarray, kernel: np.ndarray) -> np.ndarray:
    out, _ = run(input_tensor, kernel)
    return out
